# revision 2
# baseline (speedup 1.0000x reference)
"""Grouped Conv2D (32 groups of 8->8 ch, 3x3, SAME) on 8 trn2 NeuronCores.

Strategy:
  - Data-parallel over batch: 32 images / 8 cores = 4 images per core.
  - 4x4 PE-array tiling (16 concurrent 32x32 tiles): column-group cg
    streams image cg's 8-row strip; row-tile r holds block-diagonal
    weights for 4 groups (4 x [8ic x 8oc]). 16 tiles x 256 useful MACs
    = 4096 useful MACs per streamed cycle -- 2x the 2x2 (64x64) tiling,
    and the max for this 8-channel group structure.
  - Batch = same strip s of all 4 images, so image img's output always
    lands on PSUM partitions 32*img..32*img+32. Strips accumulate into
    a per-chunk SBUF buffer [128, 4, 7, 448], which makes the output
    HBM DMA contiguous per channel (6.3 KB descriptors instead of the
    896 B per-strip chunks), issued in 3 staggered waves per chunk so
    the store overlaps compute and the tail stays short.
  - Per batch: 9 taps x 16 tile-matmuls accumulate in PSUM banks 0-3
    (buffer-alternating with banks 4-7), then one engine copy
    (scalar/vector alternating) evacuates to bf16 SBUF.
  - Input streams as 3 row-slices per (chunk, image) so the first
    batch only waits on 4 x 267 KB, not 4 full images.
  - bf16 in/out over HBM (host casts + pads), fp32 PSUM accumulate.
"""

import sys

import numpy as np

if "/opt/trn_rl_repo" not in sys.path:
    sys.path.insert(0, "/opt/trn_rl_repo")

import ml_dtypes

B, C, H, W = 32, 256, 56, 56
KK = 3
GROUPS = 32
CPG = 8  # in- and out-channels per group
N_CORES = 8
BPC = B // N_CORES  # images per core
HP, WP = H + 2, W + 2  # padded image
NCHUNK = 2  # 256 channels = 2 x 128 partitions
STRIP = 8  # output rows per instance (8*56=448 <= 512 fp32/bank)
NSTRIP = H // STRIP  # 7
NTAP = KK * KK
NROW = 4  # row-tiles (4 groups of 8 channels each)
NCOL = 4  # column-groups = images per batch
GPT = 4  # groups per 32x32 tile
FD = STRIP * W  # 448 matmul free dim
WU_ROUNDS = 14  # PE warm-up waves (data-independent, start immediately)
# input row-slices: strip s needs padded rows [8s, 8s+10)
SLICES = [(0, 18), (18, 42), (42, 58)]
# output waves: after strip `trig`, store strips [s0, s1)
WAVES = [(2, 0, 3), (4, 3, 5), (6, 5, 7)]


def _pack_weights(w: np.ndarray) -> np.ndarray:
    """[256, 8, 3, 3] fp32 -> [128 pc, 2 ck, 9 tap, 32] bf16.

    wpk[32r + 8j + ic, ck, 3*th+tw, 8j + oc] = w[128ck + 32r + 8j + oc, ic, th, tw]
    """
    wr = w.reshape(NCHUNK, NROW, GPT, CPG, CPG, KK, KK)  # ck, r, j, oc, ic, th, tw
    wpk = np.zeros((NROW, GPT, CPG, NCHUNK, NTAP, GPT, CPG), dtype=np.float32)
    for j in range(GPT):
        # [ck, r, oc, ic, th, tw] -> [r, ic, ck, (th tw), oc]
        blk = wr[:, :, j].transpose(1, 3, 0, 4, 5, 2).reshape(NROW, CPG, NCHUNK, NTAP, CPG)
        wpk[:, j, :, :, :, j, :] = blk
    return wpk.reshape(128, NCHUNK, NTAP, GPT * CPG).astype(ml_dtypes.bfloat16)


def _build_bass():
    import concourse.tile as tile
    from concourse import bacc, mybir

    nc = bacc.Bacc()
    xs = nc.dram_tensor(
        "xs", [BPC, C, HP, WP], mybir.dt.bfloat16, kind="ExternalInput"
    )
    wpk = nc.dram_tensor(
        "wpk", [128, NCHUNK, NTAP, 32], mybir.dt.bfloat16, kind="ExternalInput"
    )
    out = nc.dram_tensor(
        "out", [BPC, C, H, W], mybir.dt.bfloat16, kind="ExternalOutput"
    )

    with tile.TileContext(nc) as tc:
        with (
            tc.tile_pool(name="singles", bufs=1) as singles,
            tc.tile_pool(name="xpad_pool", bufs=8) as xpad_pool,
            tc.tile_pool(name="obuf_pool", bufs=2) as obuf_pool,
            tc.tile_pool(name="psum_pool", bufs=2, space="PSUM") as psum_pool,
        ):
            # PE warm-up in the same 4x4 tiled mode as the real matmuls
            # (mode switches drain the array), on a memset scratch tile so
            # it needs no input data and starts immediately, covering the
            # HAM clock ramp while the first input and weights stream in.
            wu_src = singles.tile([128, 512], mybir.dt.bfloat16)
            nc.vector.memset(wu_src[:], 0.0)
            wu = psum_pool.tile([128, NROW, 512], mybir.dt.float32, name="ps")
            for _ in range(WU_ROUNDS):
                for r in range(NROW):
                    for cg in range(NCOL):
                        nc.tensor.matmul(
                            wu[32 * cg : 32 * cg + 32, r, :FD],
                            lhsT=wu_src[32 * r : 32 * r + 32, :32],
                            rhs=wu_src[32 * r : 32 * r + 32, :FD],
                            start=True,
                            stop=True,
                            tile_position=(32 * r, 32 * cg),
                        )

            # input tiles; slice A of chunk 0 on the two HW-DGE rings for a
            # fast path to the first batch, everything else on SW-DGE in
            # consumption order.
            xpads = {}
            for ck in range(NCHUNK):
                for img in range(BPC):
                    xpads[(ck, img)] = xpad_pool.tile(
                        [128, HP, WP], mybir.dt.bfloat16, name="xpad"
                    )
            lo, hi = SLICES[0]
            for img in range(BPC):
                xp = xpads[(0, img)]
                eng = nc.sync if img < 2 else nc.scalar
                eng.dma_start(
                    out=xp[:, lo:hi, :], in_=xs[img, 0:128, lo:hi, :]
                )
            w_sb = singles.tile([128, NCHUNK, NTAP, 32], mybir.dt.bfloat16)
            nc.scalar.dma_start(out=w_sb[:], in_=wpk[:])
            for lo, hi in SLICES[1:]:
                for img in range(BPC):
                    nc.gpsimd.dma_start(
                        out=xpads[(0, img)][:, lo:hi, :],
                        in_=xs[img, 0:128, lo:hi, :],
                    )
            for lo, hi in SLICES:
                for img in range(BPC):
                    nc.gpsimd.dma_start(
                        out=xpads[(1, img)][:, lo:hi, :],
                        in_=xs[img, 128:256, lo:hi, :],
                    )

            # 14 batches = 2 chunks x 7 strips; each batch = strip s of all
            # 4 images (image = column-group cg).
            n_batch = 0
            for ck in range(NCHUNK):
                obuf = obuf_pool.tile(
                    [128, NROW, NSTRIP, FD], mybir.dt.bfloat16, name="obuf"
                )
                for s in range(NSTRIP):
                    ps = psum_pool.tile([128, NROW, 512], mybir.dt.float32, name="ps")
                    for t in range(NTAP):
                        th, tw = divmod(t, KK)
                        for r in range(NROW):
                            lw = w_sb[32 * r : 32 * r + 32, ck, t, :]
                            for cg in range(NCOL):
                                nc.tensor.matmul(
                                    ps[32 * cg : 32 * cg + 32, r, :FD],
                                    lhsT=lw,
                                    rhs=xpads[(ck, cg)][
                                        32 * r : 32 * r + 32,
                                        s * STRIP + th : s * STRIP + th + STRIP,
                                        tw : tw + W,
                                    ],
                                    start=(t == 0),
                                    stop=(t == NTAP - 1),
                                    tile_position=(32 * r, 32 * cg),
                                )
                    # evac on scalar/vector alternating; the kernel-ending
                    # batch splits across both so the final chain is short.
                    last = ck == NCHUNK - 1 and s == NSTRIP - 1
                    if last:
                        nc.scalar.copy(out=obuf[:, 0:2, s, :], in_=ps[:, 0:2, :FD])
                        nc.vector.tensor_copy(
                            out=obuf[:, 2:4, s, :], in_=ps[:, 2:4, :FD]
                        )
                    elif n_batch % 2 == 0:
                        nc.scalar.copy(out=obuf[:, :, s, :], in_=ps[:, :, :FD])
                    else:
                        nc.vector.tensor_copy(out=obuf[:, :, s, :], in_=ps[:, :, :FD])
                    n_batch += 1
                    for trig, s0, s1 in WAVES:
                        if s != trig:
                            continue
                        for img in range(BPC):
                            dst = out[
                                img,
                                ck * 128 : (ck + 1) * 128,
                                s0 * STRIP : s1 * STRIP,
                                :,
                            ].rearrange("(r p) h w -> p r h w", r=NROW)
                            src = obuf[32 * img : 32 * img + 32, :, s0:s1, :]
                            eng = nc.scalar if img % 2 == 0 else nc.sync
                            eng.dma_start(out=dst, in_=src)
    nc.finalize()
    return nc


_CACHE = {}


def kernel(x, w, trace=False):
    from concourse.bass_utils import run_bass_kernel_spmd

    x = np.asarray(x)
    w = np.ascontiguousarray(np.asarray(w), dtype=np.float32)

    if "nc" not in _CACHE:
        _CACHE["nc"] = _build_bass()
    nc = _CACHE["nc"]

    xbf = np.zeros((B, C, HP, WP), dtype=ml_dtypes.bfloat16)
    xbf[:, :, 1 : H + 1, 1 : W + 1] = x.astype(ml_dtypes.bfloat16)
    wpk = _pack_weights(w)
    in_maps = [
        {"xs": np.ascontiguousarray(xbf[i * BPC : (i + 1) * BPC]), "wpk": wpk}
        for i in range(N_CORES)
    ]
    res = run_bass_kernel_spmd(
        nc, in_maps, core_ids=list(range(N_CORES)), trace=trace
    )
    outs = np.concatenate([res.results[i]["out"] for i in range(N_CORES)], axis=0)
    if trace:
        kernel.last_result = res
    return outs.astype(np.float32)
